# revision 1
# baseline (speedup 1.0000x reference)
"""Trainium2 Bass kernel for NonlocalSingleBlock (B=8, C=256, N=2048).

Sharding: data-parallel over batch B across the 8 NeuronCores (one batch
element per core). Per core:
  Q = wq@x+bq, K = wk@x+bk (natural [C,N] layout)
  VT = (wv@x+bv)^T computed directly as x^T @ wv^T (no on-chip transpose)
  S^T tiles [m,n] = K^T Q; scaled by host-pretransposed beta^T; exp on ACT
  (softmax without max-subtraction -- |S*beta| is bounded well under exp
  overflow)
  row-sums of exp via ones-matmul on the PE; message = VT-stationary matmuls
  MLP with BatchNorm folded into the conv weights host-side; residual add.
Attention/MLP matmuls run in bf16 (~1.5x faster per MM on HW than f32r);
the QKV projections stay f32r for accuracy. All weights are packed into one
DRAM image (single DMA). The softmax pipeline is 4-deep double-buffered
(PSUM: st x4, msg x2 banks, sums, proj) so PE overlaps the DVE multiply and
ACT exp stages.
"""

import numpy as np
import ml_dtypes

import concourse.bass as bass
import concourse.bacc as bacc
import concourse.tile as tile
import concourse.mybir as mybir
import concourse.bass_utils as bass_utils

B, C, N = 8, 256, 2048
EPS = 1e-5
F32 = mybir.dt.float32
F32R = mybir.dt.float32r
BF16 = mybir.dt.bfloat16
NB = 4          # n-blocks per core
BLK = N // NB   # 512 query columns per block
MCH = N // 128  # 16 key chunks of 128

# matmul operand dtypes (tunable): s = Q/K/S^T path, v = es/VT/sums path,
# m = MLP path. f32r keeps ~1e-4 accuracy; bf16 is ~1.5x faster per MM on HW.
DT_S = BF16
DT_V = BF16
DT_M = BF16
DT_X = F32R

_CACHE = {}


def _np_of(dt):
    return ml_dtypes.bfloat16 if dt == BF16 else np.float32


def _pack_layout(dt_v, dt_m, dt_x=None):
    dt_x = dt_x or DT_X
    """Column layout of the packed weight images. Returns (lay4, lay16, n4, n16)
    where lay*[name] = (start, ncols)."""
    entries = [
        ("wqT", 512, BF16 if dt_x in (BF16, "qk16") else F32R),
        ("wkT", 512, BF16 if dt_x in (BF16, "qk16") else F32R),
        ("wvT", 512, BF16 if dt_x == BF16 else F32R),
        ("w1T", 256, dt_m), ("w2T", 128, dt_m), ("w3T", 256, dt_m),
        ("bias", 8, F32R), ("bv", 256, F32R), ("ones", 128, dt_v),
    ]
    lay4, lay16 = {}, {}
    n4 = n16 = 0
    for name, ncols, dt in entries:
        if dt == BF16:
            lay16[name] = (n16, ncols)
            n16 += ncols
        else:
            lay4[name] = (n4, ncols)
            n4 += ncols
    return lay4, lay16, n4, max(n16, 1)


def build_nc(loop_iters=None, dt_s=None, dt_v=None, dt_m=None, dt_x=None, beta16=False):
    dt_s = dt_s or DT_S
    dt_v = dt_v or DT_V
    dt_m = dt_m or DT_M
    dt_x = dt_x or DT_X
    nc = bacc.Bacc("TRN2", target_bir_lowering=False, debug=False)

    d = {}
    d["x"] = nc.dram_tensor("x", [C, N], F32 if dt_x == BF16 else F32R,
                            kind="ExternalInput")
    if dt_x in (BF16, "qk16"):
        d["x16"] = nc.dram_tensor("x16", [C, N], BF16, kind="ExternalInput")
    d["betaT"] = nc.dram_tensor("betaT", [N, N], BF16 if beta16 else F32, kind="ExternalInput")
    # All weights/biases packed host-side into one [128, n4] f32 image
    # (single DMA; on-chip tiles are column slices of it).
    lay4, lay16, n4, n16 = _pack_layout(dt_v, dt_m, dt_x)
    d["wpack"] = nc.dram_tensor("wpack", [128, n4], F32R, kind="ExternalInput")
    if lay16:
        d["wpack16"] = nc.dram_tensor("wpack16", [128, n16], BF16,
                                      kind="ExternalInput")
    d["out"] = nc.dram_tensor("out", [C, N], F32, kind="ExternalOutput")

    from contextlib import ExitStack, nullcontext
    with tile.TileContext(nc) as tc, ExitStack() as ctx:
        P = {}
        P["consts"] = ctx.enter_context(tc.tile_pool(name="consts", bufs=1))
        P["big"] = ctx.enter_context(tc.tile_pool(name="big", bufs=1))
        P["bt"] = ctx.enter_context(tc.tile_pool(name="bt", bufs=6))
        P["es"] = ctx.enter_context(tc.tile_pool(name="es", bufs=6))
        P["sbm"] = ctx.enter_context(tc.tile_pool(name="sbm", bufs=4))
        P["msgsb"] = ctx.enter_context(tc.tile_pool(name="msgsb", bufs=2))
        P["recip"] = ctx.enter_context(tc.tile_pool(name="recip", bufs=2))
        P["h"] = ctx.enter_context(tc.tile_pool(name="h", bufs=2))
        P["outp"] = ctx.enter_context(tc.tile_pool(name="outp", bufs=4))
        # PSUM banks: st 4x1 + msg 1x2 + sums 1 + proj 1 = 8
        P["st"] = ctx.enter_context(tc.tile_pool(name="st", bufs=4, space="PSUM"))
        P["sums"] = ctx.enter_context(tc.tile_pool(name="sums", bufs=1, space="PSUM"))
        P["msg"] = ctx.enter_context(tc.tile_pool(name="msg", bufs=1, space="PSUM"))
        P["proj"] = ctx.enter_context(tc.tile_pool(name="proj", bufs=1, space="PSUM"))

        cst = _load_consts(nc, P, d, dt_s, dt_v, dt_m, dt_x)
        loop_cm = tc.For_i(0, loop_iters, 1) if loop_iters else nullcontext()
        with loop_cm:
            _emit_body(nc, tc, P, d, cst, dt_s, dt_v, dt_m, dt_x)

    nc.compile()
    return nc


def _load_consts(nc, P, d, dt_s, dt_v, dt_m, dt_x):
    consts = P["consts"]
    lay4, lay16, n4, n16 = _pack_layout(dt_v, dt_m, dt_x)
    cst = {}
    wp4 = consts.tile([128, n4], F32R, name="wp4_sb")
    head = min(1024, n4)
    nc.sync.dma_start(out=wp4[:, 0:head], in_=d["wpack"].ap()[:, 0:head])
    if n4 > head:
        nc.sync.dma_start(out=wp4[:, head:n4], in_=d["wpack"].ap()[:, head:n4])
    wp16 = None
    if lay16:
        wp16 = consts.tile([128, n16], BF16, name="wp16_sb")
        nc.sync.dma_start(out=wp16, in_=d["wpack16"].ap())

    def sl(name):
        lay, t = (lay4, wp4) if name in lay4 else (lay16, wp16)
        off, ncols = lay[name]
        return t[:, off:off + ncols]

    for nm in ("wqT", "wkT", "wvT"):
        cst[nm] = sl(nm).rearrange("p (t o) -> p t o", t=2)
    cst["w1T"] = sl("w1T").rearrange("p (t o) -> p t o", t=2)
    cst["w2T"] = sl("w2T")
    cst["w3T"] = sl("w3T")
    b = sl("bias").bitcast(F32)
    cst["bq"] = b[:, 0:2]
    cst["bk"] = b[:, 2:4]
    cst["b3"] = b[:, 4:6]
    cst["b1"] = b[:, 6:7]
    cst["b2"] = b[:, 7:8]
    cst["bv"] = sl("bv").bitcast(F32)
    cst["ones"] = sl("ones")
    return cst


def _emit_body(nc, tc, P, d, cst, dt_s, dt_v, dt_m, dt_x):
    bt_dt = d["betaT"].dtype if hasattr(d["betaT"], "dtype") else F32
    AF = mybir.ActivationFunctionType
    OP = mybir.AluOpType
    x_d, betaT_d, out_d = d["x"], d["betaT"], d["out"]

    # ---- x (4 DMAs so the first QK matmuls start early) ----
    x_sb = P["big"].tile([128, 2, N], F32 if dt_x == BF16 else F32R,
                         tag="x", name="x_sb")
    if dt_x in (BF16, "qk16"):
        xqk_sb = P["big"].tile([128, 2, N], BF16, tag="x16", name="x16_sb")
        x16_re = d["x16"].ap().rearrange("(t p) n -> p t n", p=128)
        for q in range(4):
            qs = slice(q * (N // 4), (q + 1) * (N // 4))
            nc.sync.dma_start(out=xqk_sb[:, :, qs], in_=x16_re[:, :, qs])
        nc.sync.dma_start(out=x_sb, in_=x_d.ap().rearrange("(t p) n -> p t n", p=128))
    else:
        x_re = x_d.ap().rearrange("(t p) n -> p t n", p=128)
        for q in range(4):
            qs = slice(q * (N // 4), (q + 1) * (N // 4))
            nc.sync.dma_start(out=x_sb[:, :, qs], in_=x_re[:, :, qs])
        xqk_sb = x_sb
    xv_sb = xqk_sb if dt_x == BF16 else x_sb  # VT matmul input

    # ---- Q, K: [co, n] = sum_ci wT[ci, co] x[ci, n] + b[co] ----
    q_sb = P["big"].tile([128, 2, N], dt_s, tag="q", name="q_sb")
    k_sb = P["big"].tile([128, 2, N], dt_s, tag="k", name="k_sb")
    qk_rot = ["st", "st", "st", "st", "msg", "sums", "proj"]
    qk_i = 0
    for w_sb, b_sb, dst in ((cst["wqT"], cst["bq"], q_sb), (cst["wkT"], cst["bk"], k_sb)):
        for co in range(2):
            for nb in range(NB):
                rtag = qk_rot[qk_i % len(qk_rot)]
                qk_i += 1
                ps = P[rtag].tile([128, BLK], F32, tag=rtag, name="qk_ps")
                nsl = slice(nb * BLK, (nb + 1) * BLK)
                for ci in range(2):
                    nc.tensor.matmul(
                        ps,
                        w_sb[:, ci, co * 128:(co + 1) * 128],
                        xqk_sb[:, ci, nsl],
                        start=(ci == 0), stop=(ci == 1),
                    )
                dst_ap = dst[:, co, nsl]
                if nb % 2 == 0:
                    nc.scalar.add(dst_ap, ps, b_sb[:, co:co + 1])
                else:
                    nc.vector.tensor_scalar_add(dst_ap, ps, b_sb[:, co:co + 1])

    # ---- VT[m, c] = sum_ci x[ci, m]^T wvT[ci, c] + bv ----
    vt_sb = P["big"].tile([128, MCH, C], dt_v, tag="vt", name="vt_sb")
    bvap = cst["bv"]
    bv_b = bass.AP(tensor=bvap.tensor, offset=bvap.offset,
                   ap=[bvap.ap[0], [0, 2], bvap.ap[1]])
    vt_rot = ["st", "st", "sums", "proj"]
    for mp in range(MCH // 2):
        rtag = vt_rot[mp % 4]
        ps = P[rtag].tile([128, 2, C], F32, tag=rtag, name="vt_ps")
        for j in range(2):
            mi = 2 * mp + j
            for ci in range(2):
                nc.tensor.matmul(
                    ps[:, j, :],
                    xv_sb[:, ci, mi * 128:(mi + 1) * 128],
                    cst["wvT"][:, ci, :],
                    start=(ci == 0), stop=(ci == 1),
                )
        nc.vector.tensor_add(vt_sb[:, 2 * mp:2 * mp + 2, :], ps, bv_b)

    # ---- attention + MLP per n-block ----
    for nb in range(NB):
        nsl = slice(nb * BLK, (nb + 1) * BLK)
        msg_ps = P["msg"].tile([128, 2 * BLK], F32, tag="msg", name="msg_ps")
        sums_ps = P["sums"].tile([128, BLK], F32, tag="sums", name="sums_ps")
        bts = {}
        for mp in range(MCH // 2):
            bt = P["bt"].tile([128, 2, BLK], bt_dt, tag="bt", name="bt_sb")
            nc.sync.dma_start(
                out=bt,
                in_=betaT_d.ap()[2 * mp * 128:(2 * mp + 2) * 128, nsl]
                    .rearrange("(a p) n -> p a n", p=128))
            bts[mp] = bt
        for mi in range(MCH):
            msl = slice(mi * 128, (mi + 1) * 128)
            st = P["st"].tile([128, BLK], F32, tag="st", name="st_ps")
            for ci in range(2):
                nc.tensor.matmul(
                    st,
                    k_sb[:, ci, msl],
                    q_sb[:, ci, nsl],
                    start=(ci == 0), stop=(ci == 1),
                )
            sbm = P["sbm"].tile([128, BLK], F32, tag="sbm", name="sbm_sb")
            nc.vector.tensor_mul(sbm, st, bts[mi // 2][:, mi % 2, :])
            es = P["es"].tile([128, BLK], dt_v, tag="es", name="es_sb")
            nc.scalar.activation(es, sbm, AF.Exp)
            nc.tensor.matmul(msg_ps[:, 0:BLK], vt_sb[:, mi, 0:128], es,
                             start=(mi == 0), stop=(mi == MCH - 1))
            nc.tensor.matmul(msg_ps[:, BLK:2 * BLK], vt_sb[:, mi, 128:256], es,
                             start=(mi == 0), stop=(mi == MCH - 1))
            nc.tensor.matmul(sums_ps, cst["ones"], es,
                             start=(mi == 0), stop=(mi == MCH - 1))
        recip = P["recip"].tile([128, BLK], F32, tag="recip", name="recip_sb")
        nc.vector.reciprocal(recip, sums_ps)
        msg_sb = P["msgsb"].tile([128, 2, BLK], dt_m, tag="msgsb", name="msg_sb")
        nc.vector.tensor_mul(msg_sb[:, 0, :], msg_ps[:, 0:BLK], recip)
        nc.vector.tensor_mul(msg_sb[:, 1, :], msg_ps[:, BLK:2 * BLK], recip)

        # MLP: h1 = relu(w1f@msg+b1f); h2 = relu(w2f@h1+b2f); out = x + w3@h2 + b3
        h1p = P["proj"].tile([128, BLK], F32, tag="proj", name="h1_ps")
        for ci in range(2):
            nc.tensor.matmul(h1p, cst["w1T"][:, ci, :], msg_sb[:, ci, :],
                             start=(ci == 0), stop=(ci == 1))
        h1 = P["h"].tile([128, BLK], dt_m, tag="h1", name="h1_sb")
        nc.scalar.activation(h1, h1p, AF.Relu, bias=cst["b1"][:, 0:1])
        h2p = P["proj"].tile([128, BLK], F32, tag="proj", name="h2_ps")
        nc.tensor.matmul(h2p, cst["w2T"], h1, start=True, stop=True)
        h2 = P["h"].tile([128, BLK], dt_m, tag="h2", name="h2_sb")
        nc.scalar.activation(h2, h2p, AF.Relu, bias=cst["b2"][:, 0:1])
        for co in range(2):
            h3p = P["proj"].tile([128, BLK], F32, tag="proj", name="h3_ps")
            nc.tensor.matmul(h3p, cst["w3T"][:, co * 128:(co + 1) * 128], h2,
                             start=True, stop=True)
            ob = P["outp"].tile([128, BLK], F32, tag="ob", name="ob_sb")
            nc.vector.scalar_tensor_tensor(
                out=ob, in0=h3p, scalar=cst["b3"][:, co:co + 1],
                in1=(x_sb[:, co, nsl] if dt_x == BF16 else x_sb[:, co, nsl].bitcast(F32)),
                op0=OP.add, op1=OP.add)
            nc.sync.dma_start(
                out=out_d.ap()[co * 128:(co + 1) * 128, nsl], in_=ob)


def _prep_host(inputs, dt_s=None, dt_v=None, dt_m=None, dt_x=None, beta16=False):
    """Fold BN into conv weights, pre-transpose weights, build per-core maps."""
    dt_s = dt_s or DT_S
    dt_v = dt_v or DT_V
    dt_m = dt_m or DT_M
    dt_x = dt_x or DT_X
    f = np.float32
    npm = _np_of(dt_m)
    npv = _np_of(dt_v)
    wq, bq = inputs["wq"].astype(f), inputs["bq"].astype(f)
    wk, bk = inputs["wk"].astype(f), inputs["bk"].astype(f)
    wv, bv = inputs["wv"].astype(f), inputs["bv"].astype(f)
    inv1 = inputs["g1"] / np.sqrt(inputs["v1"] + EPS)
    w1f = (inputs["w1"] * inv1[:, None]).astype(f)
    b1f = (inputs["b1"] * inv1 + inputs["be1"] - inputs["m1"] * inv1).astype(f)
    inv2 = inputs["g2"] / np.sqrt(inputs["v2"] + EPS)
    w2f = (inputs["w2"] * inv2[:, None]).astype(f)
    b2f = (inputs["b2"] * inv2 + inputs["be2"] - inputs["m2"] * inv2).astype(f)
    w3, b3 = inputs["w3"].astype(f), inputs["b3"].astype(f)

    def fold2(wT):  # [256, X] -> [128, 2*X] with t-major columns
        X = wT.shape[1]
        return wT.reshape(2, 128, X).transpose(1, 0, 2).reshape(128, 2 * X)

    lay4, lay16, n4, n16 = _pack_layout(dt_v, dt_m, dt_x)
    pack4 = np.zeros((128, n4), dtype=f)
    pack16 = np.zeros((128, n16), dtype=ml_dtypes.bfloat16)

    def put(name, arr):
        if name in lay4:
            off, ncols = lay4[name]
            pack4[:, off:off + ncols] = arr
        else:
            off, ncols = lay16[name]
            pack16[:, off:off + ncols] = arr.astype(ml_dtypes.bfloat16)

    put("wqT", fold2(wq.T))
    put("wkT", fold2(wk.T))
    put("wvT", fold2(wv.T))
    put("w1T", fold2(w1f.T))
    put("w2T", w2f.T)
    put("w3T", w3.T)
    bias_cols = np.zeros((128, 8), dtype=f)
    bias_cols[:, 0:2] = bq.reshape(2, 128).T
    bias_cols[:, 2:4] = bk.reshape(2, 128).T
    bias_cols[:, 4:6] = b3.reshape(2, 128).T
    bias_cols[:, 6] = b1f
    bias_cols[:, 7] = b2f
    put("bias", bias_cols)
    put("bv", np.tile(bv, (128, 1)))
    put("ones", np.ones((128, 128), dtype=f))
    shared = {"wpack": pack4}
    if lay16:
        shared["wpack16"] = pack16
    x = np.asarray(inputs["cors_feature"], dtype=f)
    beta = np.asarray(inputs["beta_attention"], dtype=f)
    in_maps = []
    for b in range(B):
        m = dict(shared)
        m["x"] = np.ascontiguousarray(x[b])
        if dt_x in (BF16, "qk16"):
            m["x16"] = np.ascontiguousarray(x[b]).astype(ml_dtypes.bfloat16)
        bT = np.ascontiguousarray(beta[b].T)
        m["betaT"] = bT.astype(ml_dtypes.bfloat16) if beta16 else bT
        in_maps.append(m)
    return in_maps


def kernel(**inputs) -> np.ndarray:
    if "nc" not in _CACHE:
        _CACHE["nc"] = build_nc()
    nc = _CACHE["nc"]
    in_maps = _prep_host(inputs)
    res = bass_utils.run_bass_kernel_spmd(
        nc, in_maps, core_ids=list(range(B)), trace=False)
    out = np.stack([res.results[b]["out"] for b in range(B)], axis=0)
    return out.astype(np.float32)

